# revision 23
# baseline (speedup 1.0000x reference)
"""Trainium2 Bass kernel for nn_ConvDS (2x2 pixel-unshuffle + 4x4 grouped 1x1 conv).

Reference math (scale=2, H=W=1024, no padding needed):
    xr[b,c,i,hs,ws] = x[b, c, 2*hs + i//2, 2*ws + i%2]        (i = 2*dy + dx)
    out[b, j*C + c, hs, ws] = sum_i W[j,i] * xr[b,c,i,hs,ws]

Sharding: pure data parallel over batch B=16 -> 2 images per core on 8 cores.

Fast path (exact scaled-Hadamard weights, i.e. the Haar case):
  - view each [1024, 1024] image as [512, 2048]: one SBUF partition holds an
    output row's two source rows contiguously.
  - ScalarE ACTIVATE does deinterleave (stride-2 gather) + x0.25 scale + cast
    to fp16 in one pass, producing [a|b|c|d] phase-separated fp16 data.
  - VectorE does the 4-op Hadamard butterfly on unit-stride fp16 views, which
    enables the DVE 2x packed perf mode (all operands 2-byte, step 1).
  - Output is written as fp16 (tolerance is 2e-2; fp16 error ~1e-3), halving
    output HBM traffic: per-core bytes drop 50.3MB -> 37.7MB.
  - Input DMAs issue on the Sync HWDGE ring (kept input-only: out-DMA
    descriptor generation on this ring measurably stalls the input stream).
  - Output DMAs issue via GpSimd SWDGE: the otherwise-idle engine absorbs
    the ~0.9us/DMA issue cost, and SWDGE concatenates the four 1KB j-runs
    per partition into ~3.6KB packets (~10% better packet efficiency).
  - The final block is split into 4 column slices with its out-DMAs moved to
    ScalarE's HWDGE ring, so GpSimd's ~4.5us SWDGE drain overlaps the tail.
  - Measured: ~104.5us (min of 3) vs ~133.3us baseline; the mid-phase runs
    the 16 SDMA engines ~100% packed at ~25.7GB/s each (~94% of the 435GB/s
    fabric ceiling); remaining overhead is framework preamble/teardown.

General 4x4 weights fall back to an fp32 exact path.
"""

import numpy as np

import concourse.mybir as mybir
import concourse.tile as tile
from concourse import bacc
from concourse.bass_utils import run_bass_kernel_spmd

N_CORES = 8
B, C, H, W = 16, 3, 1024, 1024
Hs, Ws = H // 2, W // 2  # 512, 512
BP = B // N_CORES  # batches per core
F32 = mybir.dt.float32
F16 = mybir.dt.float16

TILE_P = 128  # partitions (output rows hs) per block
BLK_F = 2 * W  # free dim per block: two image rows per partition
N_BLOCKS = Hs // TILE_P  # 4 row-blocks per image

# Hadamard sign rows in i = 2*dy + dx ordering (matches reference butterfly)
_HROWS = np.array(
    [
        [1.0, 1.0, 1.0, 1.0],
        [1.0, -1.0, 1.0, -1.0],
        [1.0, 1.0, -1.0, -1.0],
        [1.0, -1.0, -1.0, 1.0],
    ],
    dtype=np.float64,
)


def _match_haar(w):
    """Return the uniform positive scale s if w == s * _HROWS (identity row
    order), else None."""
    w = np.asarray(w, dtype=np.float64)
    s = w[0, 0]
    if s <= 0:
        return None
    if np.allclose(w, s * _HROWS, rtol=1e-6, atol=0):
        return float(s)
    return None


# ---------------------------------------------------------------------------
# Fast fp16 Haar path
# ---------------------------------------------------------------------------

DEFAULT_CFG = dict(
    xbufs=8,   # input f32 tile prefetch depth (8KB/partition each)
    bufs=7,    # fp16 intermediate tile buffers (4KB/partition each)
    in_eng="sync",     # engine(s) issuing input DMAs (str or tuple cycled)
    out_eng="gpsimd",  # SWDGE out: concatenates the 4x1KB j-runs into ~3.6KB
                       # packets and keeps issue off ScalarE/Sync
    deint_eng="scalar",  # deint+scale+cast rides ScalarE's free ACT affine
    tail_split=4,      # split the final block into column sub-blocks to
                       # shorten the drain-out chain at the end
    out_last_eng="scalar",  # last out-DMAs on HWDGE so GpSimd's expensive
                            # SWDGE drain overlaps the final block
    out_dtype="f16",   # "f16", or "i8": quantized int8 DRAM output, cast in
                       # the SWDGE out-DMA; host decodes by multiplying OUT_Q
                       # (measured slower than f16 — cast path defeats packet
                       # concatenation; kept for reference)
    sem_top=None,      # if set, cap the kernel semaphore pool at this ID
    tail_deint_eng=None,  # deint engine(s) for the final block's sub-units
)

# int8 output quantization step: covers |out| up to 127*OUT_Q = 2.8 with the
# reference max |out| ~2.45, while the q/2 rounding error (~0.011 abs) stays
# well inside the 2e-2 relative-max tolerance.
OUT_Q = 2.8 / 127.0


def _build_haar_fp16(scale, cfg=None):
    """Fast path program. scale: the uniform Hadamard row scale (0.25)."""
    cfg = {**DEFAULT_CFG, **(cfg or {})}
    # Optionally narrow the kernel semaphore pool (measured: no gain over
    # the default; kept as an experiment knob, off by default).
    import concourse.bass as _bass
    if not hasattr(_bass, "_orig_kernel_sem_range"):
        _bass._orig_kernel_sem_range = _bass.get_kernel_semaphore_range
    if cfg.get("sem_top"):
        _top = cfg["sem_top"]
        _bass.get_kernel_semaphore_range = (
            lambda: range(_bass.get_walrus_max_sem_num(), _top)
        )
    else:
        _bass.get_kernel_semaphore_range = _bass._orig_kernel_sem_range
    nc = bacc.Bacc(None)
    int8_out = cfg["out_dtype"] == "i8"
    odt = mybir.dt.int8 if int8_out else F16
    act_scale = scale / OUT_Q if int8_out else scale
    xd = nc.dram_tensor("x", [BP, C, Hs, BLK_F], F32, kind="ExternalInput")
    od = nc.dram_tensor("out", [BP, 4 * C, Hs, Ws], odt, kind="ExternalOutput")
    nc._out_quant = OUT_Q if int8_out else None

    def eng(spec, idx):
        if isinstance(spec, (tuple, list)):
            spec = spec[idx % len(spec)]
        return {"sync": nc.sync, "scalar": nc.scalar, "vector": nc.vector,
                "gpsimd": nc.gpsimd}[spec]

    with tile.TileContext(nc) as tc:
        with (
            tc.tile_pool(name="xp", bufs=cfg["xbufs"]) as xp,
            tc.tile_pool(name="yp", bufs=cfg["bufs"]) as yp,
            tc.tile_pool(name="tp", bufs=cfg["bufs"]) as tp,
            tc.tile_pool(name="op", bufs=cfg["bufs"]) as op,
        ):
            idx = 0

            def emit(b, c, t, w0, w1, idx, out_over=None, deint_over=None):
                """One pipeline unit: rows [t*128,(t+1)*128), out cols [w0,w1)."""
                oview = od[b].rearrange("(j c2) h w -> c2 h j w", j=4)
                rows = slice(t * TILE_P, (t + 1) * TILE_P)
                ws = w1 - w0
                X = xp.tile([TILE_P, 2 * 2 * ws], F32)
                if ws == Ws:
                    eng(cfg["in_eng"], idx).dma_start(X[:], xd[b, c, rows, :])
                else:
                    src_d = xd[b, c, rows, :].rearrange("p (h e) -> p h e", h=2)
                    eng(cfg["in_eng"], idx).dma_start(
                        X[:].rearrange("p (h e) -> p h e", h=2),
                        src_d[:, :, 2 * w0 : 2 * w1],
                    )
                # deinterleave + scale + cast -> Y = [a|b|c|d] fp16
                # source element (h, w, par) at offset h*2ws + 2w + par
                # dest   element (h, par, w) at offset h*2ws + par*ws + w
                Y = yp.tile([TILE_P, 4 * ws], F16)
                src = X[:].rearrange("p (h w par) -> p h par w", h=2, w=ws, par=2)
                dst = Y[:].rearrange("p (h par w) -> p h par w", h=2, par=2)
                de = eng(deint_over or cfg["deint_eng"], idx)
                if de is nc.scalar:
                    nc.scalar.mul(dst, src, act_scale)
                else:
                    de.tensor_scalar_mul(dst, src, act_scale)
                # stage 1: horizontal butterfly (unit-stride fp16)
                # T layout: [A+B | C+D | A-B | C-D]
                T = tp.tile([TILE_P, 4 * ws], F16)
                Y4 = Y[:].rearrange("p (x w) -> p x w", x=4)
                T4 = T[:].rearrange("p (x w) -> p x w", x=4)
                nc.vector.tensor_add(T4[:, 0:2], Y4[:, 0::2], Y4[:, 1::2])
                nc.vector.tensor_sub(T4[:, 2:4], Y4[:, 0::2], Y4[:, 1::2])
                # stage 2: vertical butterfly -> O = [o0|o1|o2|o3]
                O = op.tile([TILE_P, 4 * ws], F16)
                O4 = O[:].rearrange("p (x w) -> p x w", x=4)
                nc.vector.tensor_add(O4[:, 0:2], T4[:, 0::2], T4[:, 1::2])
                nc.vector.tensor_sub(O4[:, 2:4], T4[:, 0::2], T4[:, 1::2])
                # out: SBUF [p, (j w)] -> DRAM [h, j, w]
                eng(out_over or cfg["out_eng"], idx).dma_start(
                    oview[c, rows, :, w0:w1],
                    O[:].rearrange("p (j w) -> p j w", j=4),
                )

            units = [(b, c, t) for b in range(BP) for c in range(C)
                     for t in range(N_BLOCKS)]
            ts = cfg["tail_split"]
            for u, (b, c, t) in enumerate(units):
                last = u == len(units) - 1
                oo = cfg["out_last_eng"] if last else None
                if last and ts > 1:
                    step = Ws // ts
                    for k in range(ts):
                        emit(b, c, t, k * step, (k + 1) * step, idx,
                             out_over=oo, deint_over=cfg["tail_deint_eng"])
                        idx += 1
                else:
                    emit(b, c, t, 0, Ws, idx, out_over=oo)
                    idx += 1
    nc.compile()
    return nc


# ---------------------------------------------------------------------------
# General fp32 fallback (arbitrary 4x4 weights) — exact
# ---------------------------------------------------------------------------

def _build_general(w, bufs=6):
    nc = bacc.Bacc(None)
    xd = nc.dram_tensor("x", [BP, C, Hs, BLK_F], F32, kind="ExternalInput")
    od = nc.dram_tensor("out", [BP, 4 * C, Hs, Ws], F32, kind="ExternalOutput")

    with tile.TileContext(nc) as tc:
        with (
            tc.tile_pool(name="xp", bufs=bufs) as xp,
            tc.tile_pool(name="sp", bufs=bufs) as sp,
            tc.tile_pool(name="up", bufs=bufs) as up,
            tc.tile_pool(name="op", bufs=bufs) as op,
        ):
            for b in range(BP):
                for c in range(C):
                    oview = od[b].rearrange("(j c2) h w -> c2 h j w", j=4)
                    for t in range(N_BLOCKS):
                        X = xp.tile([TILE_P, BLK_F], F32)
                        nc.sync.dma_start(
                            X[:], xd[b, c, t * TILE_P : (t + 1) * TILE_P, :]
                        )
                        va = X[:, 0:W:2]
                        vb = X[:, 1:W:2]
                        vc = X[:, W : 2 * W : 2]
                        vd = X[:, W + 1 : 2 * W : 2]
                        O = op.tile([TILE_P, 4 * Ws], F32)
                        T = sp.tile([TILE_P, 4 * Ws], F32)
                        U = up.tile([TILE_P, 2 * Ws], F32)
                        vs = (va, vb, vc, vd)
                        for j in range(4):
                            for i in range(4):
                                nc.vector.tensor_scalar_mul(
                                    T[:, i * Ws : (i + 1) * Ws], vs[i], float(w[j, i])
                                )
                            nc.vector.tensor_add(
                                U[:, 0:Ws], T[:, 0:Ws], T[:, Ws : 2 * Ws]
                            )
                            nc.vector.tensor_add(
                                U[:, Ws : 2 * Ws],
                                T[:, 2 * Ws : 3 * Ws],
                                T[:, 3 * Ws : 4 * Ws],
                            )
                            nc.vector.tensor_add(
                                O[:, j * Ws : (j + 1) * Ws],
                                U[:, 0:Ws],
                                U[:, Ws : 2 * Ws],
                            )
                        nc.scalar.dma_start(
                            oview[c, t * TILE_P : (t + 1) * TILE_P],
                            O[:].rearrange("p (j w) -> p j w", j=4),
                        )
    nc.compile()
    return nc


_CACHE = {}


def _get_program(w, cfg=None):
    key = (w.tobytes(), repr(sorted((cfg or {}).items())))
    if key not in _CACHE:
        s = _match_haar(w)
        if s is not None:
            _CACHE[key] = _build_haar_fp16(s, cfg)
        else:
            _CACHE[key] = _build_general(w)
    return _CACHE[key]


def _run(x, conv_weights, cfg=None, **spmd_kwargs):
    x = np.ascontiguousarray(np.asarray(x, dtype=np.float32))
    w = np.asarray(conv_weights, dtype=np.float32)
    assert x.shape == (B, C, H, W), x.shape
    nc = _get_program(w, cfg)
    in_maps = [
        {"x": x[k * BP : (k + 1) * BP].reshape(BP, C, Hs, BLK_F)}
        for k in range(N_CORES)
    ]
    res = run_bass_kernel_spmd(nc, in_maps, list(range(N_CORES)), **spmd_kwargs)
    out = np.concatenate([res.results[k]["out"] for k in range(N_CORES)], axis=0)
    q = getattr(nc, "_out_quant", None)
    if q is not None:
        out = out.astype(np.float32) * np.float32(q)
    return out.astype(np.float32, copy=False), res


def kernel(x, conv_weights):
    out, _ = _run(x, conv_weights)
    return out


def kernel_timed(x, conv_weights, **spmd_kwargs):
    """Run with NTFF profiling; returns (out, BassKernelResults)."""
    return _run(x, conv_weights, trace=True, **spmd_kwargs)
